# revision 1
# baseline (speedup 1.0000x reference)
"""Trainium2 Bass kernel for nn_Castro2025Model.

Contract: kernel(**inputs) takes FULL inputs {inputs:[8192,512,8] f32,
params_raw:[13] f32} and returns the FULL output [8192,512,4] f32.
Internally: data-parallel shard over the session axis across 8 NeuronCores.

Algorithm notes (validated vs the jax reference to ~3e-7 rel):
  The per-trial Q update with full error overwrite is exactly
      q[cc] <- c_t,   c_t = rv*(1+gamma) - gamma   (exact: rewards binary)
  followed by mean-mixing  q <- alpha_t*q' + beta_t*sum(q')*1.
  With per-chunk descaling  q~ = q / prod(alpha)  the recurrence becomes
      v_t   = overwrite(q~_{t-1}, cc_t, c~_t)          [copy_predicated]
      sig_t = sum_A(v_t)                               [tensor_reduce]
      q~_t  = v_t + rho_t * sig_t                      [scalar_tensor_tensor]
  which is 3-4 DVE ops per sequential step; everything else (c~, cum, tsls,
  softmax+lapse+log, bonus) is parallel over T in big tiles on ACT/GPSIMD/DVE,
  with lapse folded into ACT immediates:
      P = exp(z + ln(1-lapse));  S = sum_A P
      logits = ln(P + lapse/(4(1-lapse))*S) - ln(S/(1-lapse))
"""

import math
import numpy as np

A = 4
NCORES = 8
PART = 128


# ---------------------------------------------------------------- host math
def _host_params(params_raw: np.ndarray) -> dict:
    p = params_raw.astype(np.float64)

    def sp(x):
        return np.log1p(np.exp(-abs(x))) + max(x, 0.0)

    def sg(x):
        return 1.0 / (1.0 + np.exp(-x))

    return dict(
        beta_r=float(np.clip(sp(p[0]), 0.01, 20.0)),
        lapse=float(np.clip(sg(p[1]), 0.01, 0.99)),
        prior=float(np.clip(sp(p[2]), 0.01, 0.99)),
        alpha=float(np.clip(sg(p[3]), 0.01, 0.99)),
        decay=float(np.clip(sg(p[4]), 0.01, 0.99)),
        ab1=float(p[5]),
        ab2=float(p[6]),
        pers=float(sp(p[7])),
        sw=float(p[8]),
        gamma=float(sp(p[10])),
        temp=float(np.clip(sp(p[11]) + 1e-6, 1e-6, 100.0)),
        beta_p=float(sp(p[12])),
    )


def _host_schedule(pr: dict, T: int) -> dict:
    """Per-step constants: exploration decay chain (fp32-faithful), alpha/rho.

    The kernel stores q^ = k*q (k = beta_r/temp) directly: the per-step
    carry-over multiplies by alpha_t, and the overwrite value is
    c^ = k*alpha_t*c, so no running Phi product (and no descaling) exists.
    """
    e = np.empty(T, np.float64)
    x = np.float32(pr["alpha"])
    for t in range(T):
        x = np.float32(x * np.float32(1.0 - 1e-3))
        e[t] = float(x)
    alph = pr["decay"] * (1.0 - e)          # alpha_t
    rho = e / (4.0 * (1.0 - e))             # rho_t
    k = pr["beta_r"] / pr["temp"]
    # constant z-shift keeping exp() in range (cancels exactly in logits)
    zbound = k * max(1.0, pr["gamma"], pr["prior"]) \
        + pr["beta_p"] * math.log(513.0)
    zshift = max(0.0, zbound - 60.0)
    return dict(e=e, alph=alph, rho=rho, k=k, zshift=zshift)


# ---------------------------------------------------------------- program
def build_program(pr: dict, B_core: int, T: int, Tc: int,
                  copy_eng: str = 'vector', split_big: bool = False):
    """Build the per-core Bass program (SPMD across cores)."""
    import concourse.bass as bass
    import concourse.bacc as bacc
    import concourse.mybir as mybir
    import concourse.tile as tile

    f32 = mybir.dt.float32
    AL = mybir.AluOpType
    AF = mybir.ActivationFunctionType
    AX = mybir.AxisListType

    S = B_core // PART                       # sessions per partition
    NCH = T // Tc
    sch = _host_schedule(pr, T)
    rho = sch["rho"]
    alph = sch["alph"]
    k = sch["k"]

    lapse = pr["lapse"]
    ln1ml = math.log(1.0 - lapse) - sch["zshift"]
    lam2 = lapse / (4.0 * (1.0 - lapse))
    inv1ml = 1.0 / (1.0 - lapse)
    pers, sw, ab1, ab2 = pr["pers"], pr["sw"], pr["ab1"], pr["ab2"]
    prior = pr["prior"]
    beta_p = pr["beta_p"]

    nc = bacc.Bacc()
    x = nc.dram_tensor("x", [B_core, T, 2 * A], f32, kind="ExternalInput")
    # host vectors replicated across partitions: [w1, w2] (c^ coefficients)
    hv = nc.dram_tensor("hv", [PART, 2 * T], f32, kind="ExternalInput")
    # scaled identity matrices for PE accumulation: [I, beta_p*I, ab1*I, ab2*I]
    hm = nc.dram_tensor("hm", [PART, 4 * PART], f32, kind="ExternalInput")
    y = nc.dram_tensor("y", [B_core, T, A], f32, kind="ExternalOutput")

    xv = x.rearrange("(p s) t c -> p s t c", p=PART)      # [128,S,T,8]
    yv = y.rearrange("(p s) t j -> p s t j", p=PART)      # [128,S,T,4]

    def regconst(v):
        v = float(v)
        if (f32, v) not in nc.const_aps.aps:
            th = nc.alloc_sbuf_tensor(
                f"uconst_{len(nc.const_aps.aps)}", [PART, 1], f32)
            nc.gpsimd.memset(th.ap(), v)
            nc.const_aps.aps[(f32, v)] = th.ap()

    with tile.TileContext(nc) as tc:
        regconst(ln1ml)
        with (
            tc.tile_pool(name="const", bufs=1) as cstp,
            tc.tile_pool(name="ri", bufs=4) as rip,
            tc.tile_pool(name="qh", bufs=4) as qhp,
            tc.tile_pool(name="sig", bufs=2) as sigp,
            tc.tile_pool(name="ctl", bufs=2) as ctlp,
            tc.tile_pool(name="cum", bufs=2) as cump,
            tc.tile_pool(name="sm", bufs=2) as smp,
            tc.tile_pool(name="big", bufs=1) as bigp,
            tc.tile_pool(name="big2", bufs=(2 if split_big else 1)) as bigp2,
            tc.tile_pool(name="out", bufs=1) as outp,
            tc.tile_pool(name="ps", bufs=1, space="PSUM") as psp,
        ):
            # constants
            hvt = cstp.tile([PART, 2 * T], f32, tag="hv")
            nc.sync.dma_start(hvt.rearrange("p (r t) -> p r t", r=2),
                              hv.rearrange("p (r t) -> p r t", r=2))
            w1 = hvt[:, 0:T]
            w2 = hvt[:, T:2 * T]
            neg1 = cstp.tile([PART, S], f32, tag="neg1")
            nc.vector.memset(neg1[:, :], -1.0)
            zer = cstp.tile([PART, Tc], f32, tag="zer")
            nc.vector.memset(zer[:, :], 0.0)
            ccar = cstp.tile([PART, 2 * S * A], f32, tag="ccar")

            prev = dict(ri=None, qh=None, code=None, tsls=None, cum=None)

            for ck in range(NCH):
                t0 = ck * Tc
                # ---------- load input chunk: layout (s, t, ch) ----------
                ri = rip.tile([PART, S * Tc * 8], f32, tag="ri")
                riv = ri.rearrange("p (s t c) -> p s t c", s=S, t=Tc, c=8)
                nc.sync.dma_start(riv[:, :, :, :], xv[:, :, t0:t0 + Tc, :])

                def a_sj(trel):                       # [128,S,A] mask at t
                    return riv[:, :, trel, 0:A]

                # ---------- c~ chunk: layout (s, t) ----------
                ctl = ctlp.tile([PART, S * Tc], f32, tag="ctl")
                ctlv = ctl.rearrange("p (s t) -> p s t", s=S)
                rvv = riv[:, :, :, A]                 # [128,S,Tc] rewards
                w1b = w1[:, t0:t0 + Tc].unsqueeze(1).broadcast_to([PART, S, Tc])
                w2b = w2[:, t0:t0 + Tc].unsqueeze(1).broadcast_to([PART, S, Tc])
                nc.vector.tensor_tensor(out=ctlv, in0=rvv, in1=w1b, op=AL.mult)
                nc.vector.tensor_tensor(out=ctlv, in0=ctlv, in1=w2b, op=AL.subtract)

                # ---------- phase 1: sequential steps ----------
                qh = qhp.tile([PART, S * Tc * A], f32, tag="qh")
                qhv = qh.rearrange("p (s t j) -> p s t j", s=S, t=Tc)
                sig = sigp.tile([PART, S * Tc], f32, tag="sig")
                sigv = sig.rearrange("p (s t) -> p s t", s=S)
                for trel in range(Tc):
                    t = t0 + trel
                    dst = qhv[:, :, trel, :]          # [128,S,A]
                    if t == 0:
                        nc.vector.memset(dst, float(k * alph[0] * prior))
                    else:
                        src = (prev["qh"][:, :, Tc - 1, :] if trel == 0
                               else qhv[:, :, trel - 1, :])
                        if copy_eng == 'scalar':
                            nc.scalar.mul(dst, src, float(alph[t]))
                        else:
                            nc.vector.tensor_scalar_mul(
                                dst, src, float(alph[t]))
                    cbc = ctlv[:, :, trel].unsqueeze(2).broadcast_to([PART, S, A])
                    nc.vector.copy_predicated(
                        out=dst, mask=a_sj(trel).bitcast(mybir.dt.int32),
                        data=cbc)
                    nc.vector.tensor_reduce(
                        out=sigv[:, :, trel], in_=dst, axis=AX.X, op=AL.add)
                    sbc = sigv[:, :, trel].unsqueeze(2).broadcast_to([PART, S, A])
                    nc.vector.scalar_tensor_tensor(
                        out=dst, in0=sbc, scalar=float(rho[t]), in1=dst,
                        op0=AL.mult, op1=AL.add)

                # ---------- cum scans (t-major views into cum tile) ----------
                cum = cump.tile([PART, S * Tc * A], f32, tag="cum")
                cumv = cum.rearrange("p (s t j) -> p s t j", s=S, t=Tc)
                for s in range(S):
                    for j in range(A):
                        d0 = riv[:, s, :, j]          # [128,Tc]
                        dstc = cumv[:, s, :, j]       # [128,Tc] strided
                        cc0 = ((ck + 1) % 2) * S * A
                        init = (0.0 if ck == 0 else
                                ccar[:, cc0 + s * A + j:cc0 + s * A + j + 1])
                        nc.vector.tensor_tensor_scan(
                            out=dstc, data0=d0, data1=zer[:, :], initial=init,
                            op0=AL.add, op1=AL.add)

                cc1 = (ck % 2) * S * A
                nc.vector.tensor_copy(
                    out=ccar[:, cc1:cc1 + S * A].rearrange(
                        "p (s j) -> p s j", s=S),
                    in_=cumv[:, :, Tc - 1, :])

                # ---------- code / same / tsls / G ----------
                code = smp.tile([PART, S * Tc], f32, tag="code")
                codev = code.rearrange("p (s t) -> p s t", s=S)
                tmp = smp.tile([PART, S * Tc], f32, tag="smtmp")
                tmpv = tmp.rearrange("p (s t) -> p s t", s=S)
                nc.vector.scalar_tensor_tensor(
                    out=tmpv, in0=riv[:, :, :, 1], scalar=2.0, in1=riv[:, :, :, 0],
                    op0=AL.mult, op1=AL.add)
                nc.vector.scalar_tensor_tensor(
                    out=codev, in0=riv[:, :, :, 3], scalar=3.0, in1=tmpv,
                    op0=AL.mult, op1=AL.add)
                same = smp.tile([PART, S * Tc], f32, tag="same")
                samev = same.rearrange("p (s t) -> p s t", s=S)
                nc.vector.tensor_tensor(
                    out=samev[:, :, 1:], in0=codev[:, :, 1:],
                    in1=codev[:, :, 0:Tc - 1], op=AL.is_equal)
                carry = (neg1[:, :].unsqueeze(2) if ck == 0
                         else prev["code"][:, :, Tc - 1].unsqueeze(2))
                nc.vector.tensor_tensor(
                    out=samev[:, :, 0:1], in0=codev[:, :, 0:1], in1=carry,
                    op=AL.is_equal)
                tsls = smp.tile([PART, S * Tc], f32, tag="tsls")
                tslsv = tsls.rearrange("p (s t) -> p s t", s=S)
                for s in range(S):
                    init = (0.0 if ck == 0
                            else prev["tsls"][:, s, Tc - 1].unsqueeze(1))
                    nc.vector.tensor_tensor_scan(
                        out=tslsv[:, s], data0=samev[:, s], data1=samev[:, s],
                        initial=init, op0=AL.mult, op1=AL.add)
                # G = sw + (pers-sw)*same + ln(1+tsls)
                gv = tmpv                              # reuse tmp as G
                nc.scalar.activation(out=gv, in_=tslsv, func=AF.Ln, bias=1.0)
                gate = smp.tile([PART, S * Tc], f32, tag="gate")
                gatev = gate.rearrange("p (s t) -> p s t", s=S)
                nc.vector.tensor_scalar(
                    out=gatev, in0=samev, scalar1=pers - sw, scalar2=sw,
                    op0=AL.mult, op1=AL.add)
                nc.vector.tensor_tensor(out=gv, in0=gv, in1=gatev, op=AL.add)

                # -------- phase 2 big passes: 3-dim APs [p, (s t), j] --------
                ST = S * Tc
                a43 = ri.rearrange("p (st c) -> p st c", c=8)[:, :, 0:A]
                z = bigp2.tile([PART, ST * A], f32, tag="z")
                z3 = z.rearrange("p (st j) -> p st j", j=A)
                # z = q^ + beta_p * ln(1+cum)   (q^ already carries k)
                nc.scalar.activation(out=z[:, :], in_=cum[:, :],
                                     func=AF.Ln, bias=1.0)
                nc.scalar.mul(z[:, :], z[:, :], beta_p)
                nc.gpsimd.tensor_tensor(
                    out=z[:, :], in0=z[:, :], in1=qh[:, :], op=AL.add)
                # P = exp(z + ln(1-lapse)); S = sum_A P
                P = bigp.tile([PART, ST * A], f32, tag="P")
                P3 = P.rearrange("p (st j) -> p st j", j=A)
                nc.scalar.activation(out=P[:, :], in_=z[:, :], func=AF.Exp,
                                     bias=ln1ml)
                Ssum = sigp.tile([PART, ST], f32, tag="Ssum")
                nc.vector.tensor_reduce(
                    out=Ssum[:, :], in_=P3, axis=AX.X, op=AL.add)
                # g = P + lam2*S ; lg = ln(g); ls2 = ln(S/(1-lapse))
                slam = sigp.tile([PART, ST], f32, tag="slam")
                nc.scalar.mul(slam[:, :], Ssum[:, :], lam2)
                sb = slam[:, :].unsqueeze(2).broadcast_to([PART, ST, A])
                g3 = P3                                 # g/lg in place of P
                nc.gpsimd.tensor_tensor(out=g3, in0=sb, in1=P3, op=AL.add)
                nc.scalar.activation(out=P[:, :], in_=P[:, :], func=AF.Ln)
                ls2 = sigp.tile([PART, ST], f32, tag="ls2")
                nc.scalar.activation(out=ls2[:, :], in_=Ssum[:, :], func=AF.Ln,
                                     scale=inv1ml)
                # f4 = lg - ls2
                lsb = ls2[:, :].unsqueeze(2).broadcast_to([PART, ST, A])
                f4_3 = z3                               # reuse z tile
                nc.gpsimd.tensor_tensor(out=f4_3, in0=g3, in1=lsb,
                                        op=AL.subtract)
                # f1 = G*a  (Pool); rest of the bonus sum on PE into PSUM:
                #   out = f4 + f1 + ab1*a_prev + ab2*rot2(a)
                Gb = tmp[:, :].unsqueeze(2).broadcast_to([PART, ST, A])
                f1f = cum
                f1_3 = cum.rearrange("p (st j) -> p st j", j=A)
                nc.gpsimd.tensor_tensor(out=f1_3, in0=Gb, in1=a43, op=AL.mult)
                f4f = z
                # rotated / t-shifted copies of a via SBUF->SBUF DMA
                ri3 = ri.rearrange("p (st c) -> p st c", c=8)
                arot = bigp.tile([PART, ST * A], f32, tag="arot")
                ar3 = arot.rearrange("p (st j) -> p st j", j=A)
                nc.scalar.mul(ar3[:, :, 2:4], ri3[:, :, 0:2], ab2)
                nc.scalar.mul(ar3[:, :, 0:2], ri3[:, :, 2:4], ab2)
                ashf = bigp.tile([PART, ST * A], f32, tag="ashf")
                as3 = ashf.rearrange("p (st j) -> p st j", j=A)
                as4 = ashf.rearrange("p (s t j) -> p s t j", s=S, t=Tc)
                # (s t)-merged shift by one trial; per-session first rows
                # bleed from the previous session and are then overwritten
                nc.scalar.mul(as3[:, 1:, :], ri3[:, 0:ST - 1, 0:A], ab1)
                if ck == 0:
                    nc.vector.memset(as4[:, :, 0, :], 0.0)
                else:
                    nc.vector.tensor_scalar_mul(
                        as4[:, :, 0, :],
                        prev["ri"].rearrange("p (s t c) -> p s t c",
                                             s=S, t=Tc, c=8)[
                            :, :, Tc - 1, 0:A], ab1)
                nc.gpsimd.tensor_tensor(out=f4f[:, :], in0=f4f[:, :],
                                        in1=f1f[:, :], op=AL.add)
                nc.gpsimd.tensor_tensor(out=f4f[:, :], in0=f4f[:, :],
                                        in1=ashf[:, :], op=AL.add)
                ot = outp.tile([PART, ST * A], f32, tag="ot")
                nc.gpsimd.tensor_tensor(out=ot[:, :], in0=f4f[:, :],
                                        in1=arot[:, :], op=AL.add)
                ost = ot.rearrange("p (s t j) -> p s t j", s=S, t=Tc)
                nc.sync.dma_start(yv[:, :, t0:t0 + Tc, :], ost)

                prev = dict(ri=ri, qh=qhv, code=codev, tsls=tslsv, cum=cumv)

    nc.compile()
    return nc


def make_hv(pr: dict, sch: dict, T: int) -> np.ndarray:
    ka = sch["k"] * sch["alph"]              # k * alpha_t
    hvrow = np.concatenate([
        ((1.0 + pr["gamma"]) * ka).astype(np.float32),
        (pr["gamma"] * ka).astype(np.float32),
    ])
    return np.broadcast_to(hvrow, (PART, 2 * T)).copy()


def make_hm(pr: dict) -> np.ndarray:
    eye = np.eye(PART, dtype=np.float32)
    return np.concatenate(
        [eye, pr["beta_p"] * eye, pr["ab1"] * eye, pr["ab2"] * eye],
        axis=1).copy()


# ---------------------------------------------------------------- entry
def kernel(inputs: np.ndarray, params_raw: np.ndarray) -> np.ndarray:
    from concourse import bass_utils

    B, T = inputs.shape[0], inputs.shape[1]
    B_core = B // NCORES
    Tc = 64 if T % 64 == 0 else T
    pr = _host_params(np.asarray(params_raw))
    sch = _host_schedule(pr, T)

    nc = build_program(pr, B_core, T, Tc, split_big=True)

    hv = make_hv(pr, sch, T)

    xs = np.ascontiguousarray(np.asarray(inputs, np.float32))
    hm = make_hm(pr)
    in_maps = [
        {"x": xs[c * B_core:(c + 1) * B_core], "hv": hv, "hm": hm}
        for c in range(NCORES)
    ]
    res = bass_utils.run_bass_kernel_spmd(
        nc, in_maps, core_ids=list(range(NCORES)))
    return np.concatenate([r["y"] for r in res.results], axis=0)



# revision 24
# speedup vs baseline: 1.1473x; 1.1473x over previous
"""Trainium2 Bass kernel for nn_Castro2025Model.

Contract: kernel(**inputs) takes FULL inputs {inputs:[8192,512,8] f32,
params_raw:[13] f32} and returns the FULL output [8192,512,4] f32.
Internally: data-parallel shard over the session axis across 8 NeuronCores.

v3 design (vs the 4-op/step baseline):
  * The per-trial recurrence runs in per-chunk DESCALED coordinates
    q~_t = q^_t / Psi_t, removing the per-step alpha multiply:
        slot_t   <- copy_predicated(slot_t, a_t, c~_t)   [slot held q~_{t-1}]
        sig_t    <- sum_A(slot_t)                        [tensor_reduce]
        slot_t+1 <- sig_t*rho_t + slot_t                 [scalar_tensor_tensor]
    3 DVE ops per step; slot_t ends as v~_t (pre-mix) and phase 2
    reconstructs q^_t = Psi_t*v~_t + (Psi_t*rho_t)*sig_t in big tiles.
  * Emission is software-pipelined: each loop round emits chunk ck's
    input/precompute, then chunk ck-1's phase-2 (Pool/ACT/PE), then chunk
    ck's DVE chain (with ck-1's row-sum/reciprocal spliced mid-chain), so
    every engine's in-order queue overlaps chunks instead of alternating.
  * cum counts: ONE affine tensor_tensor_scan per chunk over an (s,j,t)
    copy of the actions (row resets via a 0/1 constant coefficient tile).
  * softmax+lapse: r = softmax(z) via exp / row-sum / reciprocal, then
    logits = Ln((1-lapse)*r + lapse/4) in one fused ACT op.
  * bonus one-hot terms (sw*a, ab1*a_prev, ab2*rot2(a)) accumulate on the
    otherwise-idle PE as bf16 scaled-identity matmuls (one-hots are exact
    in bf16) on top of a Pool-written f32 PSUM seed (f4 + G'*a); ACT
    copies PSUM->SBUF transposing (t,s,j)->(s,t,j) for the output DMA.
  * all activations are steered to the natural_log_exp_and_others func
    table so no LoadActFuncSet churn occurs mid-kernel.
"""

import math
import numpy as np

A = 4
NCORES = 8
PART = 128


# ---------------------------------------------------------------- host math
def _host_params(params_raw: np.ndarray) -> dict:
    p = params_raw.astype(np.float64)

    def sp(x):
        return np.log1p(np.exp(-abs(x))) + max(x, 0.0)

    def sg(x):
        return 1.0 / (1.0 + np.exp(-x))

    return dict(
        beta_r=float(np.clip(sp(p[0]), 0.01, 20.0)),
        lapse=float(np.clip(sg(p[1]), 0.01, 0.99)),
        prior=float(np.clip(sp(p[2]), 0.01, 0.99)),
        alpha=float(np.clip(sg(p[3]), 0.01, 0.99)),
        decay=float(np.clip(sg(p[4]), 0.01, 0.99)),
        ab1=float(p[5]),
        ab2=float(p[6]),
        pers=float(sp(p[7])),
        sw=float(p[8]),
        gamma=float(sp(p[10])),
        temp=float(np.clip(sp(p[11]) + 1e-6, 1e-6, 100.0)),
        beta_p=float(sp(p[12])),
    )


def _host_schedule(pr: dict, T: int, Tc: int = 64) -> dict:
    """Per-step constants for the descaled recurrence.

    Stored chain values are q~_t = q^_t / Psi_t with Psi the product of
    alpha_u from the current rescale base through t.  Bases reset at chunk
    starts and whenever |log10 Psi| exceeds RESC_L10 (keeps fp32 range).
      w1[t], w2[t]: overwrite constant  c~_t = rv*w1[t] - w2[t]
      wq[t]      : Psi_t   (phase-2 rescale of v~_t)
      wy[t]      : Psi_t * rho_t  (phase-2 mixing reconstruction)
      resc[t]    : != 0 -> multiply the carry slot by this before step t
    """
    RESC_L10 = 16.0
    e = np.empty(T, np.float64)
    x = np.float32(pr["alpha"])
    for t in range(T):
        x = np.float32(x * np.float32(1.0 - 1e-3))
        e[t] = float(x)
    alph = pr["decay"] * (1.0 - e)
    rho = e / (4.0 * (1.0 - e))
    k = pr["beta_r"] / pr["temp"]
    zbound = k * max(1.0, pr["gamma"], pr["prior"]) \
        + pr["beta_p"] * math.log(513.0)
    zshift = max(0.0, zbound - 60.0)

    w1 = np.empty(T, np.float64)
    w2 = np.empty(T, np.float64)
    wq = np.empty(T, np.float64)
    wy = np.empty(T, np.float64)
    resc = np.zeros(T, np.float64)
    psi = 1.0
    for t in range(T):
        if t % Tc == 0:
            if t > 0:
                resc[t] = psi
            psi = 1.0
        elif abs(math.log10(psi)) > RESC_L10:
            resc[t] = psi
            psi = 1.0
        w1[t] = k * (1.0 + pr["gamma"]) / psi
        w2[t] = k * pr["gamma"] / psi
        psi *= alph[t]
        wq[t] = psi
        wy[t] = psi * rho[t]
    return dict(e=e, alph=alph, rho=rho, k=k, zshift=zshift,
                w1=w1, w2=w2, wq=wq, wy=wy, resc=resc)


# ---------------------------------------------------------------- program
def build_program(pr: dict, B_core: int, T: int, Tc: int, **_ignored):
    """Build the per-core Bass program (SPMD across cores)."""
    import concourse.bass as bass
    import concourse.bacc as bacc
    import concourse.mybir as mybir
    import concourse.tile as tile

    f32 = mybir.dt.float32
    bf16 = mybir.dt.bfloat16
    i32 = mybir.dt.int32
    AL = mybir.AluOpType
    AF = mybir.ActivationFunctionType
    AX = mybir.AxisListType

    S = B_core // PART                       # sessions per partition
    SA = S * A
    ST = S * Tc
    STA = ST * A
    NCH = T // Tc
    sch = _host_schedule(pr, T, Tc)
    rho = sch["rho"]
    resc = sch["resc"]
    k = sch["k"]

    lapse = pr["lapse"]
    pers, sw = pr["pers"], pr["sw"]
    prior = pr["prior"]
    beta_p = pr["beta_p"]
    zshift = sch["zshift"]

    nc = bacc.Bacc()
    x = nc.dram_tensor("x", [B_core, T, 2 * A], f32, kind="ExternalInput")
    # host vectors replicated across partitions: [w1, w2, wq, wy]
    hv = nc.dram_tensor("hv", [PART, 4 * T], f32, kind="ExternalInput")
    # bf16 scaled identities for PE accumulation: [sw*I, ab1*I, ab2*I]
    hm = nc.dram_tensor("hm", [PART, 6 * PART], bf16, kind="ExternalInput")
    hmi = nc.dram_tensor("hmi", [PART, PART], f32, kind="ExternalInput")
    y = nc.dram_tensor("y", [B_core, T, A], f32, kind="ExternalOutput")

    xv = x.rearrange("(p s) t c -> p s t c", p=PART)      # [128,S,T,8]
    yv = y.rearrange("(p s) t j -> p s t j", p=PART)      # [128,S,T,4]

    def regconst(v):
        v = float(v)
        if (f32, v) not in nc.const_aps.aps:
            th = nc.alloc_sbuf_tensor(
                f"uconst_{len(nc.const_aps.aps)}", [PART, 1], f32)
            nc.gpsimd.memset(th.ap(), v)
            nc.const_aps.aps[(f32, v)] = th.ap()

    with tile.TileContext(nc) as tc:
        regconst(1.0)
        regconst(0.0)
        regconst(-zshift)
        regconst(lapse / 4.0)
        with (
            tc.tile_pool(name="const", bufs=1) as cstp,
            tc.tile_pool(name="ri", bufs=2) as rip,
            tc.tile_pool(name="acp", bufs=2) as acpp,
            tc.tile_pool(name="qh", bufs=2) as qhp,
            tc.tile_pool(name="atr", bufs=1) as atrp,
            tc.tile_pool(name="cum", bufs=2) as cump,
            tc.tile_pool(name="sm", bufs=2) as smp,
            tc.tile_pool(name="sq", bufs=2) as sqp,
            tc.tile_pool(name="bigL", bufs=2) as bLp,
            tc.tile_pool(name="bigU", bufs=2) as bUp,
            tc.tile_pool(name="bigE", bufs=2) as bEp,
            tc.tile_pool(name="bigG", bufs=2) as bGp,
            tc.tile_pool(name="ps", bufs=2, space="PSUM") as psp,
        ):
            # ---------------- constants ----------------
            hvt = cstp.tile([PART, 4 * T], f32, tag="hv")
            nc.sync.dma_start(hvt.rearrange("p (r t) -> p r t", r=4),
                              hv.rearrange("p (r t) -> p r t", r=4))
            w1 = hvt[:, 0:T]
            w2 = hvt[:, T:2 * T]
            wq = hvt[:, 2 * T:3 * T]
            wy = hvt[:, 3 * T:4 * T]

            hmt = cstp.tile([PART, 6 * PART], bf16, tag="hm")
            nc.sync.dma_start(hmt[:, :], hm[:, :])
            mS = [hmt[:, i * PART:(i + 1) * PART] for i in range(6)]
            hmit = cstp.tile([PART, PART], f32, tag="hmi")
            nc.sync.dma_start(hmit[:, :], hmi[:, :])

            neg1 = cstp.tile([PART, S], f32, tag="neg1")
            nc.vector.memset(neg1[:, :], -1.0)
            # scan coefficient tile: 1 everywhere, 0 at each (s,j) row start
            ones0 = cstp.tile([PART, S * A * Tc], f32, tag="ones0")
            nc.vector.memset(ones0[:, :], 1.0)
            nc.vector.memset(
                ones0.rearrange("p (s j t) -> p s j t", s=S, j=A)[:, :, :, 0:1],
                0.0)
            ccar = cstp.tile([PART, SA], f32, tag="ccar")
            nc.vector.memset(ccar[:, :], 0.0)
            ccarv = ccar.rearrange("p (s j) -> p s j", s=S)
            carA = cstp.tile([PART, 3 * SA], bf16, tag="carA")
            carAv = carA.rearrange("p (h sj) -> p h sj", h=3)

            st = {}                          # per-chunk tile state

            # ================= emit helpers =================
            def emit_pre(ck):
                """Input DMA + everything phase-2-independent for chunk ck."""
                t0 = ck * Tc
                c = {}
                ri = rip.tile([PART, S * Tc * 8], f32, tag="ri")
                riv = ri.rearrange("p (s t c) -> p s t c", s=S, t=Tc, c=8)
                nc.sync.dma_start(riv[:, :, :, :], xv[:, :, t0:t0 + Tc, :])
                c["riv"] = riv

                # c~ = rv*w1 - w2, laid (s, t) -- feeds the chain first
                ctl = smp.tile([PART, ST], f32, tag="ctl")
                ctlv = ctl.rearrange("p (s t) -> p s t", s=S)
                w1b = w1[:, t0:t0 + Tc].unsqueeze(1).broadcast_to(
                    [PART, S, Tc])
                w2b = w2[:, t0:t0 + Tc].unsqueeze(1).broadcast_to(
                    [PART, S, Tc])
                nc.gpsimd.tensor_tensor(
                    out=ctlv, in0=riv[:, :, :, A], in1=w1b, op=AL.mult)
                nc.gpsimd.tensor_tensor(
                    out=ctlv, in0=ctlv, in1=w2b, op=AL.subtract)
                c["ctlv"] = ctlv

                # compact actions (t, s, j) in bf16 (exact for one-hots)
                acp = acpp.tile([PART, STA], bf16, tag="acp")
                acp4 = acp.rearrange("p (t s j) -> p t s j", t=Tc, s=S)
                for s in range(S):
                    nc.scalar.mul(acp4[:, :, s, :], riv[:, s, :, 0:A], 1.0)
                c["acp"], c["acp4"] = acp, acp4
                nc.scalar.mul(
                    carAv[:, ck % 3].rearrange("p (s j) -> p s j", s=S),
                    acp4[:, Tc - 1], 1.0)

                # actions (s, j, t) for the cum scan
                atr = atrp.tile([PART, S * A * Tc], f32, tag="atr")
                atrv = atr.rearrange("p (s j t) -> p s j t", s=S, j=A)
                for j in range(A):
                    nc.scalar.mul(atrv[:, :, j, :], riv[:, :, :, j], 1.0)
                nc.gpsimd.tensor_tensor(
                    out=atrv[:, :, :, 0], in0=atrv[:, :, :, 0],
                    in1=ccarv, op=AL.add)
                c["atr"] = atr

                # code / same / tsls / G', all laid (t, s)
                code = smp.tile([PART, ST], f32, tag="code")
                codev = code.rearrange("p (t s) -> p t s", t=Tc)
                nc.vector.scalar_tensor_tensor(
                    out=codev, in0=acp4[:, :, :, 2], scalar=2.0,
                    in1=acp4[:, :, :, 1], op0=AL.mult, op1=AL.add)
                nc.vector.scalar_tensor_tensor(
                    out=codev, in0=acp4[:, :, :, 3], scalar=3.0, in1=codev,
                    op0=AL.mult, op1=AL.add)
                same = smp.tile([PART, ST], f32, tag="same")
                samev = same.rearrange("p (t s) -> p t s", t=Tc)
                nc.vector.tensor_tensor(
                    out=samev[:, 1:], in0=codev[:, 1:],
                    in1=codev[:, 0:Tc - 1], op=AL.is_equal)
                carry = (neg1[:, :].unsqueeze(1) if ck == 0
                         else st[ck - 1]["codev"][:, Tc - 1].unsqueeze(1))
                nc.vector.tensor_tensor(
                    out=samev[:, 0:1], in0=codev[:, 0:1], in1=carry,
                    op=AL.is_equal)
                c["codev"] = codev
                tsls = smp.tile([PART, ST], f32, tag="tsls")
                tslsv = tsls.rearrange("p (t s) -> p t s", t=Tc)
                c["tslsv"] = tslsv
                c["samev"] = samev
                lnT = sqp.tile([PART, ST], f32, tag="lnT")
                c["lnT"] = lnT
                c["same"], c["tsls"], c["ctl"] = same, tsls, ctl
                st[ck] = c

            def emit_scans(ck):
                """DVE scans + (t,s)-laid G' for chunk ck (post-chain)."""
                c = st[ck]
                cum = cump.tile([PART, S * A * Tc], f32, tag="cum")
                cumv = cum.rearrange("p (s j t) -> p s j t", s=S, j=A)
                nc.vector.tensor_tensor_scan(
                    out=cum[:, :], data0=ones0[:, :], data1=c["atr"][:, :],
                    initial=0.0, op0=AL.mult, op1=AL.add)
                nc.scalar.mul(ccarv, cumv[:, :, :, Tc - 1], 1.0)
                c["cum"] = cum
                for s in range(S):
                    init = (0.0 if ck == 0
                            else st[ck - 1]["tslsv"][:, Tc - 1, s].unsqueeze(1))
                    nc.vector.tensor_tensor_scan(
                        out=c["tslsv"][:, :, s], data0=c["samev"][:, :, s],
                        data1=c["samev"][:, :, s],
                        initial=init, op0=AL.mult, op1=AL.add)
                nc.scalar.activation(out=c["lnT"][:, :], in_=c["tsls"][:, :],
                                     func=AF.Ln, bias=1.0)
                gp = sqp.tile([PART, ST], f32, tag="gp")
                nc.vector.scalar_tensor_tensor(
                    out=gp[:, :], in0=c["same"][:, :], scalar=pers - sw,
                    in1=c["lnT"][:, :], op0=AL.mult, op1=AL.add)
                c["gp"] = gp

            def emit_chain(ck, mid=None):
                """The sequential 3-op/step DVE chain for chunk ck."""
                t0 = ck * Tc
                c = st[ck]
                qh = qhp.tile([PART, (Tc + 1) * SA], f32, tag="qh")
                qhv = qh.rearrange("p (t s j) -> p t s j", t=Tc + 1, s=S)
                sig = smp.tile([PART, ST], f32, tag="sig")
                sigv = sig.rearrange("p (t s) -> p t s", t=Tc)
                c["qh"], c["qhv"], c["sigv"], c["sig"] = qh, qhv, sigv, sig
                ctlv, acp4 = c["ctlv"], c["acp4"]
                for trel in range(Tc):
                    if mid is not None and trel == 44:
                        mid()
                    t = t0 + trel
                    slot = qhv[:, trel]               # [p, S, A] contiguous
                    if t == 0:
                        nc.vector.memset(slot, float(k * prior))
                    elif trel == 0:
                        nc.vector.tensor_scalar_mul(
                            slot, st[ck - 1]["qhv"][:, Tc], float(resc[t]))
                    elif resc[t] != 0.0:
                        nc.vector.tensor_scalar_mul(
                            slot, slot, float(resc[t]))
                    cbc = ctlv[:, :, trel].unsqueeze(2).broadcast_to(
                        [PART, S, A])
                    nc.vector.copy_predicated(
                        out=slot,
                        mask=c["riv"][:, :, trel, 0:A].bitcast(i32), data=cbc)
                    nc.vector.tensor_reduce(
                        out=sigv[:, trel], in_=slot, axis=AX.X, op=AL.add)
                    sbc = sigv[:, trel].unsqueeze(2).broadcast_to(
                        [PART, S, A])
                    nc.vector.scalar_tensor_tensor(
                        out=qhv[:, trel + 1], in0=sbc,
                        scalar=float(rho[t]), in1=slot,
                        op0=AL.mult, op1=AL.add)

            def emit_p2a(ck):
                """Phase 2 of chunk ck, pre-rowsum part (Pool/ACT).

                Note: the per-step mean-mixing shift rho*sig is constant
                across actions, so it cancels inside the softmax -- z only
                needs Psi*v~ (+ beta_p*ln1p(cum)), never the mixed q^.
                """
                t0 = ck * Tc
                c = st[ck]
                # u = Psi * v~  (slots 0..Tc-1 are contiguous in (t,s,j))
                uz = bUp.tile([PART, STA], f32, tag="uz")
                uz3 = uz.rearrange("p (t sj) -> p t sj", t=Tc)
                wqb = wq[:, t0:t0 + Tc].unsqueeze(2).broadcast_to(
                    [PART, Tc, SA])
                nc.gpsimd.tensor_tensor(
                    out=uz3, in0=c["qh"].rearrange(
                        "p (t sj) -> p t sj", t=Tc + 1)[:, 0:Tc],
                    in1=wqb, op=AL.mult)
                # L = beta_p * ln(1+cum), transposed read per session
                L = bLp.tile([PART, STA], f32, tag="L")
                Lv = L.rearrange("p (t s j) -> p t s j", t=Tc, s=S)
                cumT = c["cum"].rearrange("p (s j t) -> p s t j", s=S, j=A)
                for s in range(S):
                    nc.scalar.activation(out=Lv[:, :, s, :], in_=cumT[:, s],
                                         func=AF.Ln, bias=1.0)
                nc.scalar.mul(L[:, :], L[:, :], beta_p)
                # z = u + beta_p*L ; E = exp(z - zshift)
                nc.gpsimd.tensor_tensor(
                    out=uz[:, :], in0=uz[:, :], in1=L[:, :], op=AL.add)
                E = bEp.tile([PART, STA], f32, tag="E")
                nc.scalar.activation(out=E[:, :], in_=uz[:, :], func=AF.Exp,
                                     bias=-zshift)
                c["E"] = E
                c["uz"] = uz

            def emit_rowsum(ck):
                """DVE row-sum + reciprocal for chunk ck (spliced mid-chain)."""
                c = st[ck]
                E3 = c["E"].rearrange("p (ts j) -> p ts j", j=A)
                Ss = sqp.tile([PART, ST], f32, tag="Ss")
                nc.vector.tensor_reduce(
                    out=Ss[:, :], in_=E3, axis=AX.X, op=AL.add)
                rc = sqp.tile([PART, ST], f32, tag="rc")
                nc.vector.reciprocal(out=rc[:, :], in_=Ss[:, :])
                c["rc"] = rc

            def emit_p2b(ck):
                """Phase 2 of chunk ck, post-rowsum part (Pool/ACT/PE/DMA)."""
                t0 = ck * Tc
                c = st[ck]
                E = c["E"]
                E3 = E.rearrange("p (ts j) -> p ts j", j=A)
                rcb = c["rc"][:, :].unsqueeze(2).broadcast_to([PART, ST, A])
                nc.gpsimd.tensor_tensor(out=E3, in0=E3, in1=rcb, op=AL.mult)
                # f4 = ln((1-lapse)*r + lapse/4) == ln(probs), in place
                nc.scalar.activation(out=E[:, :], in_=E[:, :], func=AF.Ln,
                                     scale=1.0 - lapse, bias=lapse / 4.0)
                # Ga = a * G'
                Ga = bGp.tile([PART, STA], f32, tag="Ga")
                gb = c["gp"][:, :].unsqueeze(2).broadcast_to([PART, ST, A])
                nc.gpsimd.tensor_tensor(
                    out=Ga.rearrange("p (ts j) -> p ts j", j=A),
                    in0=c["acp"].rearrange("p (ts j) -> p ts j", j=A),
                    in1=gb, op=AL.mult)
                # seed = f4 + Ga in SBUF, ACT writes it into PSUM; bf16
                # one-hot matmuls then accumulate on top
                nc.gpsimd.tensor_tensor(
                    out=Ga[:, :], in0=E[:, :], in1=Ga[:, :], op=AL.add)
                pt = psp.tile([PART, STA], f32, tag="pt")
                for b in range(STA // 512):
                    c0, c1 = b * 512, (b + 1) * 512
                    nc.tensor.matmul(
                        pt[:, c0:c1], hmit[:, :], Ga[:, c0:c1],
                        start=True, stop=False)
                pt3 = pt.rearrange("p (ts j) -> p ts j", j=A)
                acp = c["acp"]
                acp3 = acp.rearrange("p (ts j) -> p ts j", j=A)
                NB = STA // 512
                # hi/lo bf16 split of each constant keeps the one-hot
                # terms exact to ~1e-7 (a-values are exact in bf16)
                for h in range(2):
                    mSW, mAB1, mAB2 = mS[3 * h], mS[3 * h + 1], mS[3 * h + 2]
                    for b in range(NB):
                        c0, c1 = b * 512, (b + 1) * 512
                        nc.tensor.matmul(
                            pt[:, c0:c1], mSW, acp[:, c0:c1],
                            start=False, stop=False, skip_group_check=True)
                    for b in range(NB):
                        c0, c1 = b * 512, (b + 1) * 512
                        if b == 0:
                            nc.tensor.matmul(
                                pt[:, SA:512], mAB1, acp[:, 0:512 - SA],
                                start=False, stop=False,
                                skip_group_check=True)
                            if ck > 0:
                                nc.tensor.matmul(
                                    pt[:, 0:SA], mAB1,
                                    carAv[:, (ck - 1) % 3],
                                    start=False, stop=False,
                                    skip_group_check=True)
                        else:
                            nc.tensor.matmul(
                                pt[:, c0:c1], mAB1, acp[:, c0 - SA:c1 - SA],
                                start=False, stop=False,
                                skip_group_check=True)
                    for b in range(NB):
                        r0, r1 = b * 128, (b + 1) * 128
                        nc.tensor.matmul(
                            pt3[:, r0:r1, 0:2], mAB2, acp3[:, r0:r1, 2:4],
                            start=False, stop=False, skip_group_check=True)
                        nc.tensor.matmul(
                            pt3[:, r0:r1, 2:4], mAB2, acp3[:, r0:r1, 0:2],
                            start=False, stop=(h == 1 and b == NB - 1),
                            skip_group_check=True)
                # PSUM -> SBUF transposed to (s,t,j) (reuses Ga) -> DRAM
                ptv = pt.rearrange("p (t s j) -> p t s j", t=Tc, s=S)
                ost = Ga.rearrange("p (s t j) -> p s t j", s=S, t=Tc)
                for s in range(S):
                    nc.scalar.activation(out=ost[:, s], in_=ptv[:, :, s, :],
                                         func=AF.Copy)
                nc.sync.dma_start(yv[:, :, t0:t0 + Tc, :], ost)
                # release references that are no longer needed
                for key in ("riv", "atr", "cum", "E", "uz", "gp", "rc",
                            "acp", "acp4", "lnT", "same", "ctl", "ctlv"):
                    c.pop(key, None)

            # ================= pipelined emission =================
            for ck in range(NCH):
                emit_pre(ck)
                if ck > 0:
                    emit_p2a(ck - 1)
                mid = (lambda c0=ck - 1: emit_rowsum(c0)) if ck > 0 else None
                emit_chain(ck, mid=mid)
                emit_scans(ck)
                if ck > 0:
                    emit_p2b(ck - 1)
                if ck >= 2:
                    st.pop(ck - 2, None)
            emit_p2a(NCH - 1)
            emit_rowsum(NCH - 1)
            emit_p2b(NCH - 1)

    import concourse.bacc as bacc_mod
    orig_gat = bacc_mod.get_activation_tables

    def _gat_combined(arch):
        tabs = orig_gat(arch)
        return {name: (funcs if name == "natural_log_exp_and_others"
                       else set())
                for name, funcs in tabs.items()}

    bacc_mod.get_activation_tables = _gat_combined
    try:
        nc.compile()
    finally:
        bacc_mod.get_activation_tables = orig_gat
    return nc


def make_hv(pr: dict, sch: dict, T: int) -> np.ndarray:
    hvrow = np.concatenate([
        sch["w1"].astype(np.float32), sch["w2"].astype(np.float32),
        sch["wq"].astype(np.float32), sch["wy"].astype(np.float32)])
    return np.broadcast_to(hvrow, (PART, 4 * T)).copy()


def make_hm(pr: dict) -> np.ndarray:
    import ml_dtypes
    bf = ml_dtypes.bfloat16
    eye = np.eye(PART, dtype=np.float32)
    blocks = []
    los = []
    for key in ("sw", "ab1", "ab2"):
        v = np.float32(pr[key])
        hi = np.float32(bf(v))
        blocks.append((hi * eye).astype(bf))
        los.append((np.float32(v - hi) * eye).astype(bf))
    return np.concatenate(blocks + los, axis=1)


# ---------------------------------------------------------------- entry
def kernel(inputs: np.ndarray, params_raw: np.ndarray) -> np.ndarray:
    from concourse import bass_utils

    B, T = inputs.shape[0], inputs.shape[1]
    B_core = B // NCORES
    Tc = 64 if T % 64 == 0 else T
    pr = _host_params(np.asarray(params_raw))
    sch = _host_schedule(pr, T, Tc)

    nc = build_program(pr, B_core, T, Tc)

    hv = make_hv(pr, sch, T)
    hm = make_hm(pr)
    hmi_eye = np.eye(PART, dtype=np.float32)

    xs = np.ascontiguousarray(np.asarray(inputs, np.float32))
    in_maps = [
        {"x": xs[c * B_core:(c + 1) * B_core], "hv": hv, "hm": hm,
         "hmi": hmi_eye}
        for c in range(NCORES)
    ]
    res = bass_utils.run_bass_kernel_spmd(
        nc, in_maps, core_ids=list(range(NCORES)))
    return np.concatenate([r["y"] for r in res.results], axis=0)


# revision 27
# speedup vs baseline: 1.2614x; 1.0994x over previous
"""Trainium2 Bass kernel for nn_Castro2025Model.

Contract: kernel(**inputs) takes FULL inputs {inputs:[8192,512,8] f32,
params_raw:[13] f32} and returns the FULL output [8192,512,4] f32.
Internally: data-parallel shard over the session axis across 8 NeuronCores.

v3 design (vs the 4-op/step baseline):
  * The per-trial recurrence runs in per-chunk DESCALED coordinates
    q~_t = q^_t / Psi_t, removing the per-step alpha multiply:
        slot_t   <- copy_predicated(slot_t, a_t, c~_t)   [slot held q~_{t-1}]
        sig_t    <- sum_A(slot_t)                        [tensor_reduce]
        slot_t+1 <- sig_t*rho_t + slot_t                 [scalar_tensor_tensor]
    3 DVE ops per step; slot_t ends as v~_t (pre-mix) and phase 2
    reconstructs q^_t = Psi_t*v~_t + (Psi_t*rho_t)*sig_t in big tiles.
  * Emission is software-pipelined: each loop round emits chunk ck's
    input/precompute, then chunk ck-1's phase-2 (Pool/ACT/PE), then chunk
    ck's DVE chain (with ck-1's row-sum/reciprocal spliced mid-chain), so
    every engine's in-order queue overlaps chunks instead of alternating.
  * cum counts: ONE affine tensor_tensor_scan per chunk over an (s,j,t)
    copy of the actions (row resets via a 0/1 constant coefficient tile).
  * softmax+lapse: r = softmax(z) via exp / row-sum / reciprocal, then
    logits = Ln((1-lapse)*r + lapse/4) in one fused ACT op.
  * bonus one-hot terms (sw*a, ab1*a_prev, ab2*rot2(a)) accumulate on the
    otherwise-idle PE as bf16 scaled-identity matmuls (one-hots are exact
    in bf16) on top of a Pool-written f32 PSUM seed (f4 + G'*a); ACT
    copies PSUM->SBUF transposing (t,s,j)->(s,t,j) for the output DMA.
  * all activations are steered to the natural_log_exp_and_others func
    table so no LoadActFuncSet churn occurs mid-kernel.
"""

import math
import numpy as np

A = 4
NCORES = 8
PART = 128


# ---------------------------------------------------------------- host math
def _host_params(params_raw: np.ndarray) -> dict:
    p = params_raw.astype(np.float64)

    def sp(x):
        return np.log1p(np.exp(-abs(x))) + max(x, 0.0)

    def sg(x):
        return 1.0 / (1.0 + np.exp(-x))

    return dict(
        beta_r=float(np.clip(sp(p[0]), 0.01, 20.0)),
        lapse=float(np.clip(sg(p[1]), 0.01, 0.99)),
        prior=float(np.clip(sp(p[2]), 0.01, 0.99)),
        alpha=float(np.clip(sg(p[3]), 0.01, 0.99)),
        decay=float(np.clip(sg(p[4]), 0.01, 0.99)),
        ab1=float(p[5]),
        ab2=float(p[6]),
        pers=float(sp(p[7])),
        sw=float(p[8]),
        gamma=float(sp(p[10])),
        temp=float(np.clip(sp(p[11]) + 1e-6, 1e-6, 100.0)),
        beta_p=float(sp(p[12])),
    )


def _host_schedule(pr: dict, T: int, Tc: int = 64) -> dict:
    """Per-step constants for the descaled recurrence.

    Stored chain values are q~_t = q^_t / Psi_t with Psi the product of
    alpha_u from the current rescale base through t.  Bases reset at chunk
    starts and whenever |log10 Psi| exceeds RESC_L10 (keeps fp32 range).
      w1[t], w2[t]: overwrite constant  c~_t = rv*w1[t] - w2[t]
      wq[t]      : Psi_t   (phase-2 rescale of v~_t)
      wy[t]      : Psi_t * rho_t  (phase-2 mixing reconstruction)
      resc[t]    : != 0 -> multiply the carry slot by this before step t
    """
    RESC_L10 = 16.0
    e = np.empty(T, np.float64)
    x = np.float32(pr["alpha"])
    for t in range(T):
        x = np.float32(x * np.float32(1.0 - 1e-3))
        e[t] = float(x)
    alph = pr["decay"] * (1.0 - e)
    rho = e / (4.0 * (1.0 - e))
    k = pr["beta_r"] / pr["temp"]
    zbound = k * max(1.0, pr["gamma"], pr["prior"]) \
        + pr["beta_p"] * math.log(513.0)
    zshift = max(0.0, zbound - 60.0)

    w1 = np.empty(T, np.float64)
    w2 = np.empty(T, np.float64)
    wq = np.empty(T, np.float64)
    wy = np.empty(T, np.float64)
    resc = np.zeros(T, np.float64)
    psi = 1.0
    for t in range(T):
        if t % Tc == 0:
            if t > 0:
                resc[t] = psi
            psi = 1.0
        elif abs(math.log10(psi)) > RESC_L10:
            resc[t] = psi
            psi = 1.0
        w1[t] = k * (1.0 + pr["gamma"]) / psi
        w2[t] = k * pr["gamma"] / psi
        psi *= alph[t]
        wq[t] = psi
        wy[t] = psi * rho[t]
    return dict(e=e, alph=alph, rho=rho, k=k, zshift=zshift,
                w1=w1, w2=w2, wq=wq, wy=wy, resc=resc)


# ---------------------------------------------------------------- program
def build_program(pr: dict, B_core: int, T: int, Tc: int, **_ignored):
    """Build the per-core Bass program (SPMD across cores)."""
    import concourse.bass as bass
    import concourse.bacc as bacc
    import concourse.mybir as mybir
    import concourse.tile as tile

    f32 = mybir.dt.float32
    bf16 = mybir.dt.bfloat16
    i32 = mybir.dt.int32
    AL = mybir.AluOpType
    AF = mybir.ActivationFunctionType
    AX = mybir.AxisListType

    S = B_core // PART                       # sessions per partition
    SA = S * A
    ST = S * Tc
    STA = ST * A
    NCH = T // Tc
    sch = _host_schedule(pr, T, Tc)
    rho = sch["rho"]
    resc = sch["resc"]
    k = sch["k"]

    lapse = pr["lapse"]
    pers, sw = pr["pers"], pr["sw"]
    prior = pr["prior"]
    beta_p = pr["beta_p"]
    zshift = sch["zshift"]

    nc = bacc.Bacc()
    x = nc.dram_tensor("x", [B_core, T, 2 * A], f32, kind="ExternalInput")
    # host vectors replicated across partitions: [w1, w2, wq, wy]
    hv = nc.dram_tensor("hv", [PART, 4 * T], f32, kind="ExternalInput")
    # bf16 scaled identities for PE accumulation: [sw*I, ab1*I, ab2*I]
    hm = nc.dram_tensor("hm", [PART, 6 * PART], bf16, kind="ExternalInput")
    hmi = nc.dram_tensor("hmi", [PART, PART], f32, kind="ExternalInput")
    y = nc.dram_tensor("y", [B_core, T, A], f32, kind="ExternalOutput")

    xv = x.rearrange("(p s) t c -> p s t c", p=PART)      # [128,S,T,8]
    yv = y.rearrange("(p s) t j -> p s t j", p=PART)      # [128,S,T,4]

    def regconst(v):
        v = float(v)
        if (f32, v) not in nc.const_aps.aps:
            th = nc.alloc_sbuf_tensor(
                f"uconst_{len(nc.const_aps.aps)}", [PART, 1], f32)
            nc.gpsimd.memset(th.ap(), v)
            nc.const_aps.aps[(f32, v)] = th.ap()

    with tile.TileContext(nc) as tc:
        regconst(1.0)
        regconst(0.0)
        regconst(-zshift)
        regconst(lapse / 4.0)
        with (
            tc.tile_pool(name="const", bufs=1) as cstp,
            tc.tile_pool(name="ri", bufs=2) as rip,
            tc.tile_pool(name="acp", bufs=3) as acpp,
            tc.tile_pool(name="qh", bufs=2) as qhp,
            tc.tile_pool(name="atr", bufs=1) as atrp,
            tc.tile_pool(name="cum", bufs=2) as cump,
            tc.tile_pool(name="sm", bufs=2) as smp,
            tc.tile_pool(name="sq", bufs=2) as sqp,
            tc.tile_pool(name="bigL", bufs=2) as bLp,
            tc.tile_pool(name="bigU", bufs=2) as bUp,
            tc.tile_pool(name="bigE", bufs=2) as bEp,
            tc.tile_pool(name="bigG", bufs=2) as bGp,
            tc.tile_pool(name="ps", bufs=2, space="PSUM") as psp,
        ):
            # ---------------- constants ----------------
            hvt = cstp.tile([PART, 4 * T], f32, tag="hv")
            nc.sync.dma_start(hvt.rearrange("p (r t) -> p r t", r=4),
                              hv.rearrange("p (r t) -> p r t", r=4))
            w1 = hvt[:, 0:T]
            w2 = hvt[:, T:2 * T]
            wq = hvt[:, 2 * T:3 * T]
            wy = hvt[:, 3 * T:4 * T]

            hmt = cstp.tile([PART, 6 * PART], bf16, tag="hm")
            nc.sync.dma_start(hmt[:, :], hm[:, :])
            mS = [hmt[:, i * PART:(i + 1) * PART] for i in range(6)]
            hmit = cstp.tile([PART, PART], f32, tag="hmi")
            nc.sync.dma_start(hmit[:, :], hmi[:, :])

            neg1 = cstp.tile([PART, S], f32, tag="neg1")
            nc.vector.memset(neg1[:, :], -1.0)
            # scan coefficient tile: 1 everywhere, 0 at each (s,j) row start
            ones0 = cstp.tile([PART, S * A * Tc], f32, tag="ones0")
            nc.vector.memset(ones0[:, :], 1.0)
            nc.vector.memset(
                ones0.rearrange("p (s j t) -> p s j t", s=S, j=A)[:, :, :, 0:1],
                0.0)
            ccar = cstp.tile([PART, SA], f32, tag="ccar")
            nc.vector.memset(ccar[:, :], 0.0)
            ccarv = ccar.rearrange("p (s j) -> p s j", s=S)
            carA = cstp.tile([PART, 4 * SA], bf16, tag="carA")
            carAv = carA.rearrange("p (h sj) -> p h sj", h=4)

            st = {}                          # per-chunk tile state

            # ================= emit helpers =================
            def emit_pre(ck):
                """Input DMA + everything phase-2-independent for chunk ck."""
                t0 = ck * Tc
                c = {}
                ri = rip.tile([PART, S * Tc * 8], f32, tag="ri")
                riv = ri.rearrange("p (s t c) -> p s t c", s=S, t=Tc, c=8)
                # chunk 0: split the load so the chain can start sooner
                npiece = 4 if ck == 0 else 1
                tp = Tc // npiece
                for pc in range(npiece):
                    nc.sync.dma_start(
                        riv[:, :, pc * tp:(pc + 1) * tp, :],
                        xv[:, :, t0 + pc * tp:t0 + (pc + 1) * tp, :])

                c["riv"] = riv

                # c~ = rv*w1 - w2, laid (s, t) -- feeds the chain first
                ctl = smp.tile([PART, ST], f32, tag="ctl")
                ctlv = ctl.rearrange("p (s t) -> p s t", s=S)
                for pc in range(npiece):
                    lo, hi = pc * tp, (pc + 1) * tp
                    w1b = w1[:, t0 + lo:t0 + hi].unsqueeze(1).broadcast_to(
                        [PART, S, tp])
                    w2b = w2[:, t0 + lo:t0 + hi].unsqueeze(1).broadcast_to(
                        [PART, S, tp])
                    nc.gpsimd.tensor_tensor(
                        out=ctlv[:, :, lo:hi], in0=riv[:, :, lo:hi, A],
                        in1=w1b, op=AL.mult)
                    nc.gpsimd.tensor_tensor(
                        out=ctlv[:, :, lo:hi], in0=ctlv[:, :, lo:hi],
                        in1=w2b, op=AL.subtract)
                c["ctlv"] = ctlv

                # compact actions (t, s, j) in bf16 (exact for one-hots)
                acp = acpp.tile([PART, STA], bf16, tag="acp")
                acp4 = acp.rearrange("p (t s j) -> p t s j", t=Tc, s=S)
                for s in range(S):
                    nc.scalar.mul(acp4[:, :, s, :], riv[:, s, :, 0:A], 1.0)
                c["acp"], c["acp4"] = acp, acp4
                nc.scalar.mul(
                    carAv[:, ck % 4].rearrange("p (s j) -> p s j", s=S),
                    acp4[:, Tc - 1], 1.0)

                # actions (s, j, t) for the cum scan
                atr = atrp.tile([PART, S * A * Tc], f32, tag="atr")
                atrv = atr.rearrange("p (s j t) -> p s j t", s=S, j=A)
                for j in range(A):
                    nc.scalar.mul(atrv[:, :, j, :], riv[:, :, :, j], 1.0)
                nc.gpsimd.tensor_tensor(
                    out=atrv[:, :, :, 0], in0=atrv[:, :, :, 0],
                    in1=ccarv, op=AL.add)
                c["atr"] = atr
                st[ck] = c

            def emit_scans(ck):
                """DVE scans/code/same + (t,s)-laid G' for ck (post-chain)."""
                c = st[ck]
                acp4 = c["acp4"]
                # code / same / tsls / G', all laid (t, s)
                code = smp.tile([PART, ST], f32, tag="code")
                codev = code.rearrange("p (t s) -> p t s", t=Tc)
                nc.vector.scalar_tensor_tensor(
                    out=codev, in0=acp4[:, :, :, 2], scalar=2.0,
                    in1=acp4[:, :, :, 1], op0=AL.mult, op1=AL.add)
                nc.vector.scalar_tensor_tensor(
                    out=codev, in0=acp4[:, :, :, 3], scalar=3.0, in1=codev,
                    op0=AL.mult, op1=AL.add)
                same = smp.tile([PART, ST], f32, tag="same")
                samev = same.rearrange("p (t s) -> p t s", t=Tc)
                nc.vector.tensor_tensor(
                    out=samev[:, 1:], in0=codev[:, 1:],
                    in1=codev[:, 0:Tc - 1], op=AL.is_equal)
                carry = (neg1[:, :].unsqueeze(1) if ck == 0
                         else st[ck - 1]["codev"][:, Tc - 1].unsqueeze(1))
                nc.vector.tensor_tensor(
                    out=samev[:, 0:1], in0=codev[:, 0:1], in1=carry,
                    op=AL.is_equal)
                c["codev"] = codev
                tsls = smp.tile([PART, ST], f32, tag="tsls")
                tslsv = tsls.rearrange("p (t s) -> p t s", t=Tc)
                c["tslsv"] = tslsv
                c["samev"] = samev
                lnT = sqp.tile([PART, ST], f32, tag="lnT")
                c["lnT"] = lnT
                c["same"], c["tsls"] = same, tsls
                cum = cump.tile([PART, S * A * Tc], f32, tag="cum")
                cumv = cum.rearrange("p (s j t) -> p s j t", s=S, j=A)
                nc.vector.tensor_tensor_scan(
                    out=cum[:, :], data0=ones0[:, :], data1=c["atr"][:, :],
                    initial=0.0, op0=AL.mult, op1=AL.add)
                nc.scalar.mul(ccarv, cumv[:, :, :, Tc - 1], 1.0)
                c["cum"] = cum
                for s in range(S):
                    init = (0.0 if ck == 0
                            else st[ck - 1]["tslsv"][:, Tc - 1, s].unsqueeze(1))
                    nc.vector.tensor_tensor_scan(
                        out=c["tslsv"][:, :, s], data0=c["samev"][:, :, s],
                        data1=c["samev"][:, :, s],
                        initial=init, op0=AL.mult, op1=AL.add)
                nc.scalar.activation(out=c["lnT"][:, :], in_=c["tsls"][:, :],
                                     func=AF.Ln, bias=1.0)
                gp = sqp.tile([PART, ST], f32, tag="gp")
                nc.vector.scalar_tensor_tensor(
                    out=gp[:, :], in0=c["same"][:, :], scalar=pers - sw,
                    in1=c["lnT"][:, :], op0=AL.mult, op1=AL.add)
                c["gp"] = gp

            def emit_chain(ck, mid=None):
                """The sequential 3-op/step DVE chain for chunk ck.

                Two independent session-group streams are interleaved so
                each stream's ops execute during the other stream's SBUF
                write-ack drain (~95ns RAW stall otherwise).
                """
                t0 = ck * Tc
                c = st[ck]
                qh = qhp.tile([PART, (Tc + 1) * SA], f32, tag="qh")
                qhv = qh.rearrange("p (t s j) -> p t s j", t=Tc + 1, s=S)
                sig = smp.tile([PART, ST], f32, tag="sig")
                sigv = sig.rearrange("p (t s) -> p t s", t=Tc)
                c["qh"], c["qhv"], c["sigv"], c["sig"] = qh, qhv, sigv, sig
                ctlv, acp4 = c["ctlv"], c["acp4"]
                h = S // 2
                gsl = (slice(0, h), slice(h, S))
                for trel in range(Tc):
                    if mid is not None and trel == 56:
                        mid()
                    t = t0 + trel
                    slot = qhv[:, trel]               # [p, S, A] contiguous
                    if t == 0:
                        nc.vector.memset(slot, float(k * prior))
                    elif trel == 0:
                        nc.vector.tensor_scalar_mul(
                            slot, st[ck - 1]["qhv"][:, Tc], float(resc[t]))
                    elif resc[t] != 0.0:
                        nc.vector.tensor_scalar_mul(
                            slot, slot, float(resc[t]))
                    for g in gsl:
                        cbc = ctlv[:, g, trel].unsqueeze(2).broadcast_to(
                            [PART, h, A])
                        nc.vector.copy_predicated(
                            out=qhv[:, trel, g],
                            mask=c["riv"][:, g, trel, 0:A].bitcast(i32),
                            data=cbc)
                    for g in gsl:
                        nc.vector.tensor_reduce(
                            out=sigv[:, trel, g], in_=qhv[:, trel, g],
                            axis=AX.X, op=AL.add)
                    for g in gsl:
                        sbc = sigv[:, trel, g].unsqueeze(2).broadcast_to(
                            [PART, h, A])
                        nc.vector.scalar_tensor_tensor(
                            out=qhv[:, trel + 1, g], in0=sbc,
                            scalar=float(rho[t]), in1=qhv[:, trel, g],
                            op0=AL.mult, op1=AL.add)

            def emit_p2a(ck):
                """Phase 2 of chunk ck, pre-rowsum part (Pool/ACT).

                Note: the per-step mean-mixing shift rho*sig is constant
                across actions, so it cancels inside the softmax -- z only
                needs Psi*v~ (+ beta_p*ln1p(cum)), never the mixed q^.
                """
                t0 = ck * Tc
                c = st[ck]
                # u = Psi * v~  (slots 0..Tc-1 are contiguous in (t,s,j))
                uz = bUp.tile([PART, STA], f32, tag="uz")
                uz3 = uz.rearrange("p (t sj) -> p t sj", t=Tc)
                wqb = wq[:, t0:t0 + Tc].unsqueeze(2).broadcast_to(
                    [PART, Tc, SA])
                nc.gpsimd.tensor_tensor(
                    out=uz3, in0=c["qh"].rearrange(
                        "p (t sj) -> p t sj", t=Tc + 1)[:, 0:Tc],
                    in1=wqb, op=AL.mult)
                # L = beta_p * ln(1+cum), transposed read per session
                L = bLp.tile([PART, STA], f32, tag="L")
                Lv = L.rearrange("p (t s j) -> p t s j", t=Tc, s=S)
                cumT = c["cum"].rearrange("p (s j t) -> p s t j", s=S, j=A)
                for s in range(S):
                    nc.scalar.activation(out=Lv[:, :, s, :], in_=cumT[:, s],
                                         func=AF.Ln, bias=1.0)
                nc.scalar.mul(L[:, :], L[:, :], beta_p)
                # z = u + beta_p*L ; E = exp(z - zshift)
                nc.gpsimd.tensor_tensor(
                    out=uz[:, :], in0=uz[:, :], in1=L[:, :], op=AL.add)
                E = bEp.tile([PART, STA], f32, tag="E")
                nc.scalar.activation(out=E[:, :], in_=uz[:, :], func=AF.Exp,
                                     bias=-zshift)
                c["E"] = E
                c["uz"] = uz

            def emit_rowsum(ck):
                """DVE row-sum + reciprocal for chunk ck (spliced mid-chain)."""
                c = st[ck]
                E3 = c["E"].rearrange("p (ts j) -> p ts j", j=A)
                Ss = sqp.tile([PART, ST], f32, tag="Ss")
                nc.vector.tensor_reduce(
                    out=Ss[:, :], in_=E3, axis=AX.X, op=AL.add)
                rc = sqp.tile([PART, ST], f32, tag="rc")
                nc.vector.reciprocal(out=rc[:, :], in_=Ss[:, :])
                c["rc"] = rc

            def emit_p2b(ck):
                """Phase 2 of chunk ck, post-rowsum part (Pool/ACT/PE/DMA)."""
                t0 = ck * Tc
                c = st[ck]
                E = c["E"]
                E3 = E.rearrange("p (ts j) -> p ts j", j=A)
                rcb = c["rc"][:, :].unsqueeze(2).broadcast_to([PART, ST, A])
                nc.gpsimd.tensor_tensor(out=E3, in0=E3, in1=rcb, op=AL.mult)
                # f4 = ln((1-lapse)*r + lapse/4) == ln(probs), in place
                nc.scalar.activation(out=E[:, :], in_=E[:, :], func=AF.Ln,
                                     scale=1.0 - lapse, bias=lapse / 4.0)
                # Ga = a * G'
                Ga = bGp.tile([PART, STA], f32, tag="Ga")
                gb = c["gp"][:, :].unsqueeze(2).broadcast_to([PART, ST, A])
                nc.gpsimd.tensor_tensor(
                    out=Ga.rearrange("p (ts j) -> p ts j", j=A),
                    in0=c["acp"].rearrange("p (ts j) -> p ts j", j=A),
                    in1=gb, op=AL.mult)
                # seed = f4 + Ga in SBUF, ACT writes it into PSUM; bf16
                # one-hot matmuls then accumulate on top
                nc.gpsimd.tensor_tensor(
                    out=Ga[:, :], in0=E[:, :], in1=Ga[:, :], op=AL.add)
                pt = psp.tile([PART, STA], f32, tag="pt")
                for b in range(STA // 512):
                    c0, c1 = b * 512, (b + 1) * 512
                    nc.tensor.matmul(
                        pt[:, c0:c1], hmit[:, :], Ga[:, c0:c1],
                        start=True, stop=False)
                pt3 = pt.rearrange("p (ts j) -> p ts j", j=A)
                acp = c["acp"]
                acp3 = acp.rearrange("p (ts j) -> p ts j", j=A)
                NB = STA // 512
                # hi/lo bf16 split of each constant keeps the one-hot
                # terms exact to ~1e-7 (a-values are exact in bf16)
                for h in range(1):
                    mSW, mAB1, mAB2 = mS[3 * h], mS[3 * h + 1], mS[3 * h + 2]
                    for b in range(NB):
                        c0, c1 = b * 512, (b + 1) * 512
                        nc.tensor.matmul(
                            pt[:, c0:c1], mSW, acp[:, c0:c1],
                            start=False, stop=False, skip_group_check=True)
                    for b in range(NB):
                        c0, c1 = b * 512, (b + 1) * 512
                        if b == 0:
                            nc.tensor.matmul(
                                pt[:, SA:512], mAB1, acp[:, 0:512 - SA],
                                start=False, stop=False,
                                skip_group_check=True)
                            if ck > 0:
                                nc.tensor.matmul(
                                    pt[:, 0:SA], mAB1,
                                    carAv[:, (ck - 1) % 4],
                                    start=False, stop=False,
                                    skip_group_check=True)
                        else:
                            nc.tensor.matmul(
                                pt[:, c0:c1], mAB1, acp[:, c0 - SA:c1 - SA],
                                start=False, stop=False,
                                skip_group_check=True)
                    for b in range(NB):
                        r0, r1 = b * 128, (b + 1) * 128
                        nc.tensor.matmul(
                            pt3[:, r0:r1, 0:2], mAB2, acp3[:, r0:r1, 2:4],
                            start=False, stop=False, skip_group_check=True)
                        nc.tensor.matmul(
                            pt3[:, r0:r1, 2:4], mAB2, acp3[:, r0:r1, 0:2],
                            start=False, stop=(h == 1 and b == NB - 1),
                            skip_group_check=True)
                # PSUM -> SBUF transposed to (s,t,j) (reuses Ga) -> DRAM
                ptv = pt.rearrange("p (t s j) -> p t s j", t=Tc, s=S)
                ost = Ga.rearrange("p (s t j) -> p s t j", s=S, t=Tc)
                for s in range(S):
                    nc.scalar.activation(out=ost[:, s], in_=ptv[:, :, s, :],
                                         func=AF.Copy)
                nc.sync.dma_start(yv[:, :, t0:t0 + Tc, :], ost)
                # release references that are no longer needed
                for key in ("riv", "atr", "cum", "E", "uz", "gp", "rc",
                            "acp", "acp4", "lnT", "same", "ctl", "ctlv"):
                    c.pop(key, None)

            # ================= pipelined emission =================
            for ck in range(NCH):
                emit_pre(ck)
                if ck >= 2:
                    emit_p2b(ck - 2)
                if ck > 0:
                    emit_p2a(ck - 1)
                mid = (lambda c0=ck - 1: emit_rowsum(c0)) if ck > 0 else None
                emit_chain(ck, mid=mid)
                emit_scans(ck)
                if ck >= 3:
                    st.pop(ck - 3, None)
            emit_p2b(NCH - 2)
            emit_p2a(NCH - 1)
            emit_rowsum(NCH - 1)
            emit_p2b(NCH - 1)

    import concourse.bacc as bacc_mod
    orig_gat = bacc_mod.get_activation_tables

    def _gat_combined(arch):
        tabs = orig_gat(arch)
        return {name: (funcs if name == "natural_log_exp_and_others"
                       else set())
                for name, funcs in tabs.items()}

    bacc_mod.get_activation_tables = _gat_combined
    try:
        nc.compile()
    finally:
        bacc_mod.get_activation_tables = orig_gat
    return nc


def make_hv(pr: dict, sch: dict, T: int) -> np.ndarray:
    hvrow = np.concatenate([
        sch["w1"].astype(np.float32), sch["w2"].astype(np.float32),
        sch["wq"].astype(np.float32), sch["wy"].astype(np.float32)])
    return np.broadcast_to(hvrow, (PART, 4 * T)).copy()


def make_hm(pr: dict) -> np.ndarray:
    import ml_dtypes
    bf = ml_dtypes.bfloat16
    eye = np.eye(PART, dtype=np.float32)
    blocks = []
    los = []
    for key in ("sw", "ab1", "ab2"):
        v = np.float32(pr[key])
        hi = np.float32(bf(v))
        blocks.append((hi * eye).astype(bf))
        los.append((np.float32(v - hi) * eye).astype(bf))
    return np.concatenate(blocks + los, axis=1)


# ---------------------------------------------------------------- entry
def kernel(inputs: np.ndarray, params_raw: np.ndarray) -> np.ndarray:
    from concourse import bass_utils

    B, T = inputs.shape[0], inputs.shape[1]
    B_core = B // NCORES
    Tc = 64 if T % 64 == 0 else T
    pr = _host_params(np.asarray(params_raw))
    sch = _host_schedule(pr, T, Tc)

    nc = build_program(pr, B_core, T, Tc)

    hv = make_hv(pr, sch, T)
    hm = make_hm(pr)
    hmi_eye = np.eye(PART, dtype=np.float32)

    xs = np.ascontiguousarray(np.asarray(inputs, np.float32))
    in_maps = [
        {"x": xs[c * B_core:(c + 1) * B_core], "hv": hv, "hm": hm,
         "hmi": hmi_eye}
        for c in range(NCORES)
    ]
    res = bass_utils.run_bass_kernel_spmd(
        nc, in_maps, core_ids=list(range(NCORES)))
    return np.concatenate([r["y"] for r in res.results], axis=0)
